# revision 1
# baseline (speedup 1.0000x reference)
"""Causal self-attention (B=4, T=2048, C=1024, H=16, D=64) on 8 trn2 NeuronCores.

Sharding: core c = (batch b=c//2, head-group hg=c%2 of 8 heads / 512 channels).
Each core computes attention for its 8 heads on its batch plus the partial
output projection over its 512 channels of Wp; the host sums the two partial
projections per batch and adds bp.

Per-core layout is feature-major ("transposed"): x is sent as xT (C, T) so
q/k project directly as qT = Wq.T @ x.T with both operands k(partition)-major.
v is computed in natural (T, D) orientation with a ones-column appended per
head so that the yT = [v|1].T @ P^T matmul also yields softmax row sums.
Matmul operands are bf16 (1 cyc/row on the PE); accumulation, softmax
internals and the final output stay fp32.

All attention matmuls are geometrically FULL 128x128-array ops (kT stored
twice per pair with complementary zero halves; v blocks padded to 128 wide)
so the PE HAM activity monitor keeps the clock gate at K=8/8 (2.4 GHz)
instead of dropping to 4/8 on the 64-row/65-col attention shapes.

Schedule: phase V (v for all heads, DMA-paced by sequence-half x loads) ->
QK(pair 0) m-outer -> per pair p: attention (software-pipelined per head:
yT matmuls of key-tile j-1 interleave between the S-matmul sections of
key-tile j), with independent full-array PE work streamed one item per
S-section into the PE's exp-wait gaps: the QK projection of pair p+1 for
p<3, and the output projection (gated on incremental per-chunk softmax
normalization) for p=3 and the tail.  The ACT engine runs only the softmax
exp during attention; all steady-state evictions go through the DVE.
"""

import math
from collections import deque

import numpy as np

B, T, C = 4, 2048, 1024
H, D = 16, 64
NCORES = 8
PAIRS = 4          # head pairs per core (2 heads = 128 channels each)
KT = C // 128      # 8 k-tiles over input channels
MT = T // 128      # 16 tiles over sequence
SC = 1.0 / math.sqrt(D)

_CACHE = {}


def _build_nc():
    from contextlib import ExitStack

    import concourse.bacc as bacc
    import concourse.mybir as mybir
    import concourse.tile as tile

    f32 = mybir.dt.float32
    bf16 = mybir.dt.bfloat16
    AF = mybir.ActivationFunctionType

    nc = bacc.Bacc("TRN2", target_bir_lowering=False, debug=False)

    xT = nc.dram_tensor("xT", (C, T), bf16, kind="ExternalInput").ap()
    wqD = nc.dram_tensor("wq", (C, 512), bf16, kind="ExternalInput").ap()
    wkD = nc.dram_tensor("wk", (C, 512), bf16, kind="ExternalInput").ap()
    wvD = nc.dram_tensor("wv", (C, 512), bf16, kind="ExternalInput").ap()
    wpD = nc.dram_tensor("wp", (512, C), bf16, kind="ExternalInput").ap()
    bqD = nc.dram_tensor("bq", (512,), f32, kind="ExternalInput").ap()
    bkD = nc.dram_tensor("bk", (512,), f32, kind="ExternalInput").ap()
    bvD = nc.dram_tensor("bv", (512,), f32, kind="ExternalInput").ap()
    # partial projections leave the core in bf16: halves the 8MB writeback
    # (it is ring-bandwidth-bound in the drain tail); the host sums the two
    # per-batch partials in fp32.
    outD = nc.dram_tensor("out", (T, C), bf16, kind="ExternalOutput").ap()

    with tile.TileContext(nc) as tc, ExitStack() as ctx:
        const = ctx.enter_context(tc.tile_pool(name="const", bufs=1))
        xp = ctx.enter_context(tc.tile_pool(name="xp", bufs=1))

        wv_sb = const.tile([128, KT, 512], bf16)
        xsb = [xp.tile([128, T], bf16, name=f"xsb{k}") for k in range(KT)]
        wq_sb = const.tile([128, KT, 512], bf16)
        wk_sb = const.tile([128, KT, 512], bf16)
        wp_sb = const.tile([128, 4, C], bf16)

        # DMA issue order = first-needed first.  V-phase t-group 0 needs only
        # (wv slice k, xsb[k] cols 0:1024); the second sequence halves and the
        # remaining weights stream in behind while the PE is already busy.
        # Alternate big transfers across both HWDGE rings (SP + ACT).
        bq_sb = const.tile([128, PAIRS], f32)
        nc.sync.dma_start(bq_sb[:], bqD.rearrange("(a p) -> p a", p=128))
        bk_sb = const.tile([128, PAIRS], f32)
        nc.scalar.dma_start(bk_sb[:], bkD.rearrange("(a p) -> p a", p=128))
        bv_row = const.tile([1, 512], f32)
        nc.sync.dma_start(bv_row[:], bvD.rearrange("(a n) -> a n", a=1))
        bv_bc = const.tile([128, 512], f32)
        nc.gpsimd.partition_broadcast(bv_bc[:], bv_row[:])
        wv4 = wvD.rearrange("(k p) n -> p k n", p=128)
        for k in range(KT):
            exs = nc.sync if k % 2 == 0 else nc.scalar
            ewv = nc.scalar if k % 2 == 0 else nc.sync
            ewv.dma_start(wv_sb[:, k, :], wv4[:, k, :])
            exs.dma_start(xsb[k][:, 0:1024], xT[k * 128:(k + 1) * 128, 0:1024])
        wq4 = wqD.rearrange("(k p) n -> p k n", p=128)
        nc.scalar.dma_start(wq_sb[:, 0:4, :], wq4[:, 0:4, :])
        nc.sync.dma_start(wq_sb[:, 4:8, :], wq4[:, 4:8, :])
        for k in range(KT):
            exs = nc.scalar if k % 2 == 0 else nc.sync
            exs.dma_start(
                xsb[k][:, 1024:2048], xT[k * 128:(k + 1) * 128, 1024:2048]
            )
        wk4 = wkD.rearrange("(k p) n -> p k n", p=128)
        nc.scalar.dma_start(wk_sb[:, 0:4, :], wk4[:, 0:4, :])
        nc.sync.dma_start(wk_sb[:, 4:8, :], wk4[:, 4:8, :])
        wp4 = wpD.rearrange("(k p) n -> p k n", p=128)
        nc.scalar.dma_start(wp_sb[:, 0:2, :], wp4[:, 0:2, :])
        nc.sync.dma_start(wp_sb[:, 2:4, :], wp4[:, 2:4, :])

        # 128x128 lower-block mask: keep (1.0) where i >= j, else 0.
        mask_tri = const.tile([128, 128], bf16)
        nc.gpsimd.memset(mask_tri[:], 1.0)
        nc.gpsimd.affine_select(
            out=mask_tri[:],
            in_=mask_tri[:],
            compare_op=mybir.AluOpType.is_ge,
            fill=0.0,
            base=0,
            pattern=[[1, 128]],
            channel_multiplier=-1,
        )

        # v for all heads, natural (t, d) layout, 128-wide blocks per head:
        # cols 0:64 = v, col 64 = ones (row-sum trick), cols 65:128 = zeros.
        # Full-width weight loads keep the PE HAM activity monitor at K=8/8.
        v_all = const.tile([128, MT * 8 * 128], bf16)
        nc.vector.memset(v_all[:], 0.0)
        v4 = v_all.rearrange("p (t h e) -> p t h e", t=MT, h=8)
        nc.gpsimd.memset(v4[:, :, :, 64:65], 1.0)

        # q^T for all 8 heads (bf16, 4KB/part each pair tile).
        qT_t = [const.tile([128, T], bf16, name=f"qT{p}") for p in range(PAIRS)]
        # k^T stored twice per pair with complementary zeroed halves so the
        # S matmul loads full 128-row weights (HAM sees a full array) while
        # streaming the fully-real shared qT pair tile.
        kT0_t = [const.tile([128, T], bf16, name=f"kT0{p}") for p in range(PAIRS)]
        kT1_t = [const.tile([128, T], bf16, name=f"kT1{p}") for p in range(PAIRS)]
        for p in range(PAIRS):
            nc.gpsimd.memset(kT0_t[p][64:128, :], 0.0)
            nc.vector.memset(kT1_t[p][0:64, :], 0.0)
        yT_tiles = [const.tile([128, T], bf16, name=f"yT{i}") for i in range(PAIRS)]

        # ---------------- Phase V + QK(0), DMA-aware interleave -------------
        # V t-group 0 and QK0's m=0,1 tiles touch only the first sequence
        # halves of x, so they run while the second halves stream in; V
        # t-group 1 and QK0 m=2,3 follow.  All share one 8-slot PSUM pool.
        gpsum = tc.alloc_tile_pool(name="gpsum", bufs=8, space="PSUM")

        def v_group(tg):
            ps = [
                gpsum.tile([128, 512], f32, tag="gp", name=f"vps{tg}_{t}")
                for t in range(8)
            ]
            for k in range(KT):
                for t8 in range(8):
                    tt = tg * 8 + t8
                    nc.tensor.matmul(
                        ps[t8][:],
                        lhsT=xsb[k][:, tt * 128:(tt + 1) * 128],
                        rhs=wv_sb[:, k, :],
                        start=(k == 0),
                        stop=(k == KT - 1),
                    )
            for t8 in range(8):
                tt = tg * 8 + t8
                nc.vector.tensor_add(
                    v4[:, tt, :, 0:64],
                    ps[t8].rearrange("p (h e) -> p h e", h=8),
                    bv_bc.rearrange("p (h e) -> p h e", h=8),
                )

        def qk0_group(qk, m):
            ms = slice(m * 512, (m + 1) * 512)
            w_sb = wq_sb if qk == 0 else wk_sb
            ps = gpsum.tile([128, 512], f32, tag="gp", name=f"qk0_{qk}_{m}")
            for k in range(KT):
                nc.tensor.matmul(
                    ps[:],
                    lhsT=w_sb[:, k, 0:128],
                    rhs=xsb[k][:, ms],
                    start=(k == 0),
                    stop=(k == KT - 1),
                )
            if qk == 0:
                nc.vector.tensor_scalar_add(
                    qT_t[0][:, ms], ps[:], bq_sb[:, 0:1]
                )
            else:
                # ACT is idle during this phase; use it for k evictions
                nc.scalar.activation(
                    kT0_t[0][0:64, ms], ps[0:64, :],
                    AF.Identity, bias=bk_sb[0:64, 0:1],
                )
                nc.scalar.activation(
                    kT1_t[0][64:128, ms], ps[64:128, :],
                    AF.Identity, bias=bk_sb[64:128, 0:1],
                )

        # Ordered by DMA arrival: wq lands before the x second halves, which
        # land before wk — so all q-projections run between the two V groups
        # and the k-projections close the phase.  No block starts before its
        # inputs arrive, so the PE never idles into a HAM re-throttle.
        v_group(0)
        for m in range(4):
            qk0_group(0, m)
        v_group(1)
        for m in range(4):
            qk0_group(1, m)
        gpsum.release()

        # ---------------- Attention with filler-slot pipelining -------------
        # One filler item is emitted into the PE queue after each S-section:
        # QK matmuls of pair p+1 during pair p<3, output-projection work
        # during pair 3 (gated on incremental normalization) and the tail.
        ptp = ctx.enter_context(tc.tile_pool(name="ptp", bufs=3))
        nrm = ctx.enter_context(tc.tile_pool(name="nrm", bufs=3))
        ostp = ctx.enter_context(tc.tile_pool(name="ost", bufs=3))
        sps = ctx.enter_context(tc.tile_pool(name="sps", bufs=2, space="PSUM"))
        yps = ctx.enter_context(tc.tile_pool(name="yps", bufs=4, space="PSUM"))
        qkp = ctx.enter_context(tc.tile_pool(name="qkp", bufs=2, space="PSUM"))

        fill_iters = deque()

        def sprinkle():
            while fill_iters:
                th = next(fill_iters[0], None)
                if th is None:
                    fill_iters.popleft()
                    continue
                th()
                return

        def drain():
            while fill_iters:
                sprinkle()

        def qk_gen(p):
            """Yield one-instruction thunks computing qT/kT for pair p."""
            for qk in range(2):
                w_sb = wq_sb if qk == 0 else wk_sb
                for m in range(4):
                    qk_ps = qkp.tile(
                        [128, 512], f32, tag="qk", name=f"qk{p}_{qk}_{m}"
                    )
                    for k in range(KT):
                        def mm(qk_ps=qk_ps, k=k, m=m, w_sb=w_sb):
                            nc.tensor.matmul(
                                qk_ps[:],
                                lhsT=w_sb[:, k, p * 128:(p + 1) * 128],
                                rhs=xsb[k][:, m * 512:(m + 1) * 512],
                                start=(k == 0),
                                stop=(k == KT - 1),
                            )
                        yield mm

                    def evict(qk_ps=qk_ps, qk=qk, m=m):
                        ms = slice(m * 512, (m + 1) * 512)
                        if qk == 0:
                            nc.vector.tensor_scalar_add(
                                qT_t[p][:, ms], qk_ps[:], bq_sb[:, p:p + 1]
                            )
                        else:
                            nc.vector.tensor_scalar_add(
                                kT0_t[p][0:64, ms], qk_ps[0:64, :],
                                bk_sb[0:64, p:p + 1],
                            )
                            nc.vector.tensor_scalar_add(
                                kT1_t[p][64:128, ms], qk_ps[64:128, :],
                                bk_sb[64:128, p:p + 1],
                            )
                    yield evict

        def proj_gen(ic):
            """Output projection for sequence tiles 4*ic..4*ic+3; valid once
            every head's chunk ic is normalized into yT_tiles."""
            for mt in range(4 * ic, 4 * ic + 4):
                ost = ostp.tile([128, C], bf16, tag="ost", name=f"ost{mt}")
                for oh in range(2):
                    pps = qkp.tile(
                        [128, 512], f32, tag="qk", name=f"pp{mt}_{oh}"
                    )
                    for k in range(4):
                        def mm(pps=pps, k=k, mt=mt, oh=oh):
                            nc.tensor.matmul(
                                pps[:],
                                lhsT=yT_tiles[k][:, mt * 128:(mt + 1) * 128],
                                rhs=wp_sb[:, k, oh * 512:(oh + 1) * 512],
                                start=(k == 0),
                                stop=(k == 3),
                            )
                        yield mm

                    def evict(pps=pps, ost=ost, mt=mt, oh=oh):
                        # DVE for one half, ACT for the other: in the drain
                        # tail exp is done and ACT is free, so the eviction
                        # latency never gates the qkp bank rotation.  DMA
                        # each half as soon as it lands so the writeback
                        # drains concurrently with the remaining matmuls.
                        if oh == 0:
                            nc.vector.tensor_copy(
                                ost[:, 0:512], pps[:]
                            )
                        else:
                            nc.scalar.activation(
                                ost[:, 512:1024], pps[:], AF.Copy
                            )
                        (nc.sync if (mt * 2 + oh) % 2 == 0
                         else nc.scalar).dma_start(
                            outD[mt * 128:(mt + 1) * 128,
                                 oh * 512:(oh + 1) * 512],
                            ost[:, oh * 512:(oh + 1) * 512],
                        )
                    yield evict

        if True:
            for p in range(PAIRS):
                if p < PAIRS - 1:
                    fill_iters.append(qk_gen(p + 1))
                qT = qT_t[p]
                for hh in range(2):
                    h = p * 2 + hh
                    kTt = kT0_t[p] if hh == 0 else kT1_t[p]
                    hs = slice(hh * 64, hh * 64 + 64)
                    ypt = [
                        yps.tile([128, 512], f32, tag="yps", name=f"y{p}_{hh}_{ic}")
                        for ic in range(4)
                    ]

                    def yt_chunks(jj):
                        out = []
                        for ic in range(jj // 4, 4):
                            a = max(ic * 512, 128 * jj)
                            out.append((ic, a, (ic + 1) * 512 - a))
                        return out

                    pending = None  # (jj, PT, chunks)
                    for j in range(MT + 1):
                        if j < MT:
                            W = T - 128 * j
                            PT = ptp.tile(
                                [128, T], bf16, tag="pt", name=f"pt{p}_{hh}_{j}"
                            )
                            nsec = (W + 511) // 512
                            for s in range(nsec):
                                sw = min(512, W - s * 512)
                                ps = sps.tile(
                                    [128, 512], f32, tag="sps",
                                    name=f"s{p}_{hh}_{j}_{s}"
                                )
                                io = 128 * j + s * 512
                                nc.tensor.matmul(
                                    ps[:, 0:sw],
                                    lhsT=kTt[:, j * 128:(j + 1) * 128],
                                    rhs=qT[:, io:io + sw],
                                    start=True,
                                    stop=True,
                                )
                                # interleave ~half the pending yT matmuls
                                # between S sections to keep the PE fed
                                if pending is not None and s == 0:
                                    jj, PTj, chunks = pending
                                    take = chunks[:max(1, len(chunks) // 2)]
                                    rest = chunks[len(take):]
                                    for ic, a, w2 in take:
                                        nc.tensor.matmul(
                                            ypt[ic][:, a - ic * 512:512],
                                            lhsT=v4[:, jj, h, :],
                                            rhs=PTj[:, a - 128 * jj:
                                                    a - 128 * jj + w2],
                                            start=(jj == 0),
                                            stop=(jj == 4 * ic + 3),
                                        )
                                    pending = (jj, PTj, rest)
                                sprinkle()
                                nc.scalar.activation(
                                    PT[:, s * 512:s * 512 + sw],
                                    ps[:, 0:sw],
                                    AF.Exp,
                                    scale=SC,
                                )
                            # zero upper-triangular part of the diagonal block
                            nc.vector.tensor_mul(
                                PT[:, 0:128], PT[:, 0:128], mask_tri[:]
                            )
                        if pending is not None:
                            jj, PTj, chunks = pending
                            for ic, a, w2 in chunks:
                                nc.tensor.matmul(
                                    ypt[ic][:, a - ic * 512:512],
                                    lhsT=v4[:, jj, h, :],
                                    rhs=PTj[:, a - 128 * jj:a - 128 * jj + w2],
                                    start=(jj == 0),
                                    stop=(jj == 4 * ic + 3),
                                )
                        # chunk ic's accumulation closes with row 4*ic+3
                        # (drained above at j == 4*ic+4): normalize it now so
                        # ypt banks free early and, on the last head, the
                        # output projection for its sequence tiles can start.
                        if j > 0 and j % 4 == 0:
                            ic = j // 4 - 1
                            sums = nrm.tile([1, 512], f32, tag="sums",
                                            name=f"sm{p}_{hh}_{ic}")
                            nc.vector.tensor_copy(sums[:], ypt[ic][64:65, :])
                            rcp_row = nrm.tile([1, 512], f32, tag="rrow",
                                               name=f"rr{p}_{hh}_{ic}")
                            nc.vector.reciprocal_approx_fast(
                                rcp_row[:], sums[:]
                            )
                            rcp = nrm.tile([64, 512], f32, tag="rcp",
                                           name=f"rc{p}_{hh}_{ic}")
                            nc.gpsimd.partition_broadcast(rcp[:], rcp_row[:])
                            nc.vector.tensor_mul(
                                yT_tiles[p][hs, ic * 512:(ic + 1) * 512],
                                ypt[ic][0:64, :],
                                rcp[:],
                            )
                            if p == PAIRS - 1 and hh == 1:
                                fill_iters.append(proj_gen(ic))
                        if j < MT:
                            pending = (j, PT, yt_chunks(j))
                # ensure pair p+1's qT/kT (and trailing proj work) are fully
                # emitted before the next pair's S sections enter the queue
                drain()

    nc.compile()
    return nc


def _get_nc():
    if "nc" not in _CACHE:
        _CACHE["nc"] = _build_nc()
    return _CACHE["nc"]


def make_in_maps(x, Wq, bq, Wk, bk, Wv, bv, Wp, bp):
    import ml_dtypes

    bf = ml_dtypes.bfloat16
    x = np.asarray(x, np.float32)
    Wq = np.asarray(Wq, np.float32).astype(bf)
    Wk = np.asarray(Wk, np.float32).astype(bf)
    Wv = np.asarray(Wv, np.float32).astype(bf)
    Wp = np.asarray(Wp, np.float32).astype(bf)
    bq = np.asarray(bq, np.float32)
    bk = np.asarray(bk, np.float32)
    bv = np.asarray(bv, np.float32)
    in_maps = []
    for c in range(NCORES):
        b, hg = divmod(c, 2)
        sl = slice(hg * 512, (hg + 1) * 512)
        in_maps.append({
            "xT": np.ascontiguousarray(x[b].T.astype(bf)),
            "wq": np.ascontiguousarray(Wq[:, sl]),
            "wk": np.ascontiguousarray(Wk[:, sl]),
            "wv": np.ascontiguousarray(Wv[:, sl]),
            "wp": np.ascontiguousarray(Wp[sl, :]),
            "bq": np.ascontiguousarray(bq[sl]),
            "bk": np.ascontiguousarray(bk[sl]),
            "bv": np.ascontiguousarray(bv[sl]),
        })
    return in_maps


def combine(results, bp):
    bp = np.asarray(bp, np.float32)
    out = np.empty((B, T, C), np.float32)
    for b in range(B):
        out[b] = (np.asarray(results[2 * b]["out"], np.float32)
                  + np.asarray(results[2 * b + 1]["out"], np.float32) + bp)
    return out


def kernel(x, Wq, bq, Wk, bk, Wv, bv, Wp, bp):
    from concourse import bass_utils

    nc = _get_nc()
    in_maps = make_in_maps(x, Wq, bq, Wk, bk, Wv, bv, Wp, bp)
    res = bass_utils.run_bass_kernel_spmd(nc, in_maps, core_ids=list(range(NCORES)))
    return combine(res.results, bp)

